# revision 34
# baseline (speedup 1.0000x reference)
"""DenseGATConv Bass/Tile kernel for Trainium2, SPMD over 8 NeuronCores.

Problem (B=4, N=2048, F=128, H=4, C=64):
  xh = (x @ W).reshape(B,N,H,C)
  a_src[b,j,h] = xh . att_src ; a_dst[b,i,h] = xh . att_dst
  s = a_src[j] + a_dst[i];  alpha = softmax_j(mask(adj+I, leaky_relu(s, 0.2)))
  out[b,i] = concat_h(sum_j alpha * xh[b,j,h,:]) + bias

Algebra (no exp over the N*N*H grid, no softmax-normalizer subtraction):
  exp(lrelu(s)) = max(exp(s), exp(0.2 s));  dividing by exp(a_dst_i)
  (cancels in softmax) gives the grid weight
    T[j,h,i] = max(Q'_i * e2_j, e1_j),  Q' = exp(-0.8 a_dst),
    e2 = exp(0.2 a_src), e1 = exp(a_src)
  G = T * adjT (self loops host-added).  PE accumulates num|den with one
  stationary load per (tile, head):
    acc[h][c,i] += xh1[j, c|1]^T @ G[j, h, i]   (f16, f32 PSUM)
  row 64 of acc is the softmax denominator (ones column in xh1).

All grid elementwise work runs on DVE (T ops ~0.45ns/elem, mask-mults
~0.55ns/elem, both in the f16 2x mode); offloading mask-mults to the
Pool engine was measured to SLOW both engines ~2.4x (shared SBUF
ports), so POOL_HEADS defaults to 0,0.  Phase A is f16 end to end (PE 1
cycle/row instead of 4).  The per-core q slice is
made core-independent by rolling each core's own 1024 destination
columns to the front of xT (and reordering adjT rows identically - the
contraction order is internal).  The softmax scalars (e1/e2 per source
node, Q' per destination) are host-folded like the weight folds: expv
is a 64KB input and Q' arrives as an 8KB row that a PE ones
outer-product broadcasts to all partitions, so the grid stream gates
only on small leading DMAs.  The destination permutation (i' = kk*128+p
<-> i = p*4+kk within 512-blocks) is applied host-side to qrow/adjT, so
the output DMA writes 4 consecutive rows per partition.

Epilogue: evacuate PSUM (ACT/DVE alternating), PE-transpose [65,128]
blocks, reciprocal + one multiply per (k,h) on DVE, f16 output DMA
(host upcasts).  Bias ops are only emitted when bias is nonzero.
"""

import os

import numpy as np

import concourse.bacc as bacc
import concourse.bass as bass
import concourse.tile as tile
from concourse import mybir
from concourse.bass_utils import run_bass_kernel_spmd
from concourse.masks import make_identity

B, N, F = 4, 2048, 128
H, C = 4, 64
HC = H * C
TBUFS = int(os.environ.get('TBUFS', 5))
GBUFS = int(os.environ.get('GBUFS', 6))
ABUFS = int(os.environ.get('ABUFS', 3))
POOL_HEADS = os.environ.get('POOL_HEADS', '1,2')  # heads for pool on even,odd tiles
N_CORES = 8
ID = N // 2          # dest rows per core
NT = N // 128        # 16 source tiles
NKD = ID // 512      # 2 dest 512-chunks
F32 = mybir.dt.float32
F16 = mybir.dt.float16

_NC_CACHE = {}


def build_nc(reps: int = 1, has_bias: bool = False):
    nc = bacc.Bacc("TRN2", target_bir_lowering=False, debug=False, num_devices=1)

    d_xT = nc.dram_tensor("xT", [F, N], F16, kind="ExternalInput").ap()
    d_adjT = nc.dram_tensor("adjT", [NT, 128, ID], F16, kind="ExternalInput").ap()
    d_wcat = nc.dram_tensor("Wcat", [F, HC], F16, kind="ExternalInput").ap()
    d_expv = nc.dram_tensor("expvin", [128, NT, 8], F32, kind="ExternalInput").ap()
    d_qrow = nc.dram_tensor("qrow", [1, H * ID], F16, kind="ExternalInput").ap()
    d_bias = nc.dram_tensor("biasv", [1, HC], F32, kind="ExternalInput").ap()
    d_out = nc.dram_tensor("out", [ID, HC], F16, kind="ExternalOutput").ap()

    EXP = mybir.ActivationFunctionType.Exp
    CPY = mybir.ActivationFunctionType.Copy

    pool_even_n = int(POOL_HEADS.split(',')[0])
    pool_odd_n = int(POOL_HEADS.split(',')[1])

    with tile.TileContext(nc) as tc:
        with tc.tile_pool(name="const", bufs=1) as const:
            ident = const.tile([128, 128], F32)
            make_identity(nc, ident)
            ones1 = const.tile([1, 128], F32)
            nc.vector.memset(ones1, 1.0)

            # preload the activation table set while input DMAs run
            scratch1 = const.tile([1, 4], F32)
            nc.scalar.activation(scratch1, ones1[0:1, 0:4], CPY)
            ones1h = const.tile([1, 128], F16)
            nc.vector.memset(ones1h, 1.0)

            xT = const.tile([F, N], F16)
            wcat = const.tile([F, HC], F16)
            qrow_sb = const.tile([1, H * ID], F16)
            expv = const.tile([128, NT, 8], F32)
            q_bc = const.tile([128, H, ID], F16)        # Q' broadcast per head
            nc.sync.dma_start(out=qrow_sb, in_=d_qrow)
            nc.sync.dma_start(out=expv, in_=d_expv)
            # heads 0/1: DMA partition-broadcast straight from the input row
            # (lands before the PE chain); heads 2/3 via PE outer-product
            for h in range(2):
                hrow = d_qrow[0:1, h * ID:(h + 1) * ID]
                for pc in range(4):
                    q_bcast = bass.AP(
                        tensor=hrow.tensor, offset=hrow.offset,
                        ap=[[0, 32]] + list(hrow.ap[1:]))
                    nc.sync.dma_start(out=q_bc[pc * 32:(pc + 1) * 32, h, :],
                                      in_=q_bcast)
            nc.sync.dma_start(out=wcat, in_=d_wcat)
            nc.sync.dma_start(out=xT[:, 0:1024], in_=d_xT[:, 0:1024])
            nc.sync.dma_start(out=xT[:, 1024:2048], in_=d_xT[:, 1024:2048])
            bias_sb = const.tile([1, HC], F32)
            if has_bias:
                nc.sync.dma_start(out=bias_sb, in_=d_bias)

            # persistent per-core tensors
            xh1 = const.tile([128, NT, H, 65], F16)     # [xh | 1] per (t,h)
            bias_bc = const.tile([128, HC], F32)

            # ---------------- phase A: projections ----------------
            with tc.tile_pool(name="psA", bufs=2, space="PSUM") as psA, \
                 tc.tile_pool(name="psQ", bufs=2, space="PSUM") as psQp, \
                 tc.tile_pool(name="psB", bufs=1, space="PSUM") as psBp:
                sc_a = nc.enter_named_scope("phA", False)
                # grid scalars and Q' come host-precomputed.  Q' is an 8KB
                # row DMA broadcast on-chip by a PE ones outer-product, so
                # the grid start does not wait on 1MB of broadcast DMA.
                for h in range(2, H):
                    qb_ps = psQp.tile([128, ID], F32)
                    for kb in range(NKD):
                        nc.tensor.matmul(
                            qb_ps[:, kb * 512:(kb + 1) * 512], ones1h,
                            qrow_sb[0:1, h * ID + kb * 512:h * ID + (kb + 1) * 512],
                            start=True, stop=True)
                    nc.scalar.activation(q_bc[:, h, :], qb_ps, CPY)
                nc.gpsimd.memset(xh1[:, :, :, 64:65], 1.0)
                # projection tiles; grid tile t can start once tile t is done
                for t in range(NT):
                    ps = psA.tile([128, HC], F32)
                    nc.tensor.matmul(ps, xT[:, t * 128:(t + 1) * 128], wcat,
                                     start=True, stop=True)
                    # raw xh into the 65-column head blocks
                    nc.scalar.activation(xh1[:, t, :, 0:64], ps, CPY)
                if has_bias:
                    psb2 = psBp.tile([128, HC], F32, tag="psbias", bufs=1)
                    nc.tensor.matmul(psb2, ones1, bias_sb, start=True, stop=True)
                    nc.scalar.activation(bias_bc, psb2, CPY)
                nc.leave_named_scope("phA", sc_a[0], False)

            # ---------------- phase B: grid + matmul accumulate ----------------
            with tc.tile_pool(name="ep_sb", bufs=1) as epsb:
                with tc.tile_pool(name="acc", bufs=1, space="PSUM") as accp:
                    acc = {}
                    for h in range(H):
                        acc_t = accp.tile([65, ID], F32, tag=f"acc{h}",
                                          name=f"acc{h}")
                        acc[h] = acc_t

                    sc_b = nc.enter_named_scope("phB", False)
                    with tc.tile_pool(name="adj", bufs=ABUFS) as adjp, \
                         tc.tile_pool(name="grid", bufs=4) as gridp:
                        for rep in range(reps):
                            for t in range(NT):
                                adjt = adjp.tile([128, ID], F16)
                                nc.sync.dma_start(out=adjt, in_=d_adjT[t])
                                t_all = gridp.tile([128, H, ID], F16, tag="T", bufs=TBUFS)
                                g = gridp.tile([128, H, ID], F16, tag="G", bufs=GBUFS)
                                adj_rep2 = bass.AP(
                                    tensor=adjt.tensor, offset=adjt.offset,
                                    ap=[adjt.ap[0], [0, 2]] + list(adjt.ap[1:]))
                                first = (rep == 0 and t == 0)
                                last = (rep == reps - 1 and t == NT - 1)
                                # head pairs: ts,ts,tt then the pair's matmuls,
                                # so PE starts 2 heads earlier on each tile
                                for hp2 in range(2):
                                    for h in (2 * hp2, 2 * hp2 + 1):
                                        # T = max(Q' * exp(.2 a_src), exp(a_src))
                                        nc.vector.tensor_scalar(
                                            out=t_all[:, h, :], in0=q_bc[:, h, :],
                                            scalar1=expv[:, t, h:h + 1],
                                            scalar2=expv[:, t, 4 + h:5 + h],
                                            op0=mybir.AluOpType.mult,
                                            op1=mybir.AluOpType.max)
                                    nc.vector.tensor_tensor(
                                        out=g[:, 2 * hp2:2 * hp2 + 2, :],
                                        in0=t_all[:, 2 * hp2:2 * hp2 + 2, :],
                                        in1=adj_rep2, op=mybir.AluOpType.mult)
                                    for h in (2 * hp2, 2 * hp2 + 1):
                                        for k in range(NKD):
                                            nc.tensor.matmul(
                                                acc[h][:, k * 512:(k + 1) * 512],
                                                xh1[:, t, h, :],
                                                g[:, h, k * 512:(k + 1) * 512],
                                                start=first, stop=last)

                    nc.leave_named_scope("phB", sc_b[0], False)
                    sc_c = nc.enter_named_scope("phC", False)
                    # evacuate accumulators to SBUF (ACT is close to PSUM)
                    s_tiles = {}
                    for h in range(H):
                        s = epsb.tile([65, ID], F32, tag=f"s{h}", name=f"s{h}")
                        if h % 2 == 0:
                            nc.scalar.activation(s, acc[h], CPY)
                        else:
                            nc.vector.tensor_copy(s, acc[h])
                        for k in range(NKD):
                            s_tiles[(h, k)] = s[:, k * 512:(k + 1) * 512]

                # acc PSUM released here
                # ------------- phase C: transpose + divide (+bias) + out -------------
                with tc.tile_pool(name="ep_ps", bufs=4, space="PSUM") as epps, \
                     tc.tile_pool(name="ep_sm", bufs=8) as epsm, \
                     tc.tile_pool(name="outp", bufs=2) as outp:
                    for k in range(NKD):
                        osb = outp.tile([128, 4, HC], F16, tag="osb", name="osb")
                        for h in range(H):
                            pt = epps.tile([128, 4, 65], F32)
                            for kk in range(4):
                                nc.tensor.transpose(
                                    pt[:, kk, :],
                                    s_tiles[(h, k)][:, kk * 128:(kk + 1) * 128],
                                    ident[0:65, 0:65])
                            rec = epsm.tile([128, 4, 1], F32)
                            nc.vector.reciprocal(rec, pt[:, :, 64:65])
                            rec_rep = bass.AP(
                                tensor=rec.tensor, offset=rec.offset,
                                ap=[rec.ap[0], rec.ap[1], [0, 64]])
                            nc.vector.tensor_tensor(
                                out=osb[:, :, h * 64:(h + 1) * 64],
                                in0=pt[:, :, 0:64], in1=rec_rep,
                                op=mybir.AluOpType.mult)
                            if has_bias:
                                bias_rep = bass.AP(
                                    tensor=bias_bc.tensor,
                                    offset=bias_bc.offset + h * 64,
                                    ap=[bias_bc.ap[0], [0, 4], [1, 64]])
                                nc.vector.tensor_tensor(
                                    out=osb[:, :, h * 64:(h + 1) * 64],
                                    in0=osb[:, :, h * 64:(h + 1) * 64],
                                    in1=bias_rep,
                                    op=mybir.AluOpType.add)
                        # destination rows are host-permuted so partition p
                        # holds 4 consecutive output rows (contiguous 2KB f16
                        # descriptor per partition)
                        blk = d_out[k * 512:(k + 1) * 512, :]
                        out_ap = bass.AP(
                            tensor=blk.tensor, offset=blk.offset,
                            ap=[[4 * HC, 128], [HC, 4], [1, HC]])
                        nc.sync.dma_start(out=out_ap, in_=osb)
                    nc.leave_named_scope("phC", sc_c[0], False)

    nc.compile()
    return nc


def _get_nc(reps: int = 1, has_bias: bool = False):
    key = (reps, has_bias)
    if key not in _NC_CACHE:
        _NC_CACHE[key] = build_nc(reps, has_bias)
    return _NC_CACHE[key]


def make_in_maps(x, adj, W, att_src, att_dst, bias):
    x = np.asarray(x, dtype=np.float32)
    adj = np.asarray(adj, dtype=np.float32)
    W = np.asarray(W, dtype=np.float32)
    att_src = np.asarray(att_src, dtype=np.float32)
    att_dst = np.asarray(att_dst, dtype=np.float32)
    bias = np.asarray(bias, dtype=np.float32)

    # weight prep: fold per-head attention dots into projection columns
    wa_src = np.stack([W[:, h * C:(h + 1) * C] @ att_src[h] for h in range(H)], 1)
    wa_dst = np.stack([W[:, h * C:(h + 1) * C] @ att_dst[h] for h in range(H)], 1)
    wcat = np.ascontiguousarray(W, dtype=np.float16)               # [F, 256]

    adjl = adj.copy()
    idx = np.arange(N)
    adjl[:, idx, idx] = 1.0

    # destination-row permutation: kernel position i' = kk*128 + p within each
    # 512-block maps to original row p*4 + kk, so the output DMA writes
    # contiguous per-partition chunks (the q-path moving AP applies the same
    # permutation on-device)
    perm = np.concatenate([kb * 512 + (np.arange(512) % 128) * 4 + np.arange(512) // 128
                           for kb in range(ID // 512)])

    in_maps = []
    for c in range(N_CORES):
        b, half = c // 2, c % 2
        # roll the core's own destination columns to the front so the shared
        # program's q slice (cols 0:ID) is core-independent; reorder adjT's
        # source rows identically (contraction order is internal)
        order = np.concatenate([np.arange(half * ID, (half + 1) * ID),
                                np.arange((1 - half) * ID, (2 - half) * ID)])
        xT = np.ascontiguousarray(x[b].T[:, order], dtype=np.float16)
        adjT = np.ascontiguousarray(
            adjl[b].T[order, :][:, half * ID + perm]).astype(np.float16)
        # softmax scalars, host-folded: e2|e1 per source node (rolled order),
        # Q' per destination node (permuted order)
        a_src = (x[b] @ wa_src)[order]
        expvin = np.concatenate([np.exp(0.2 * a_src), np.exp(a_src)],
                                axis=1).astype(np.float32)
        expvin = np.ascontiguousarray(
            expvin.reshape(NT, 128, 8).transpose(1, 0, 2))
        a_dst = (x[b] @ wa_dst)[half * ID + perm]
        qrow = np.ascontiguousarray(
            np.exp(-0.8 * a_dst).T.astype(np.float16)).reshape(1, H * ID)
        in_maps.append({
            "xT": xT,
            "adjT": adjT.reshape(NT, 128, ID),
            "Wcat": wcat,
            "expvin": expvin,
            "qrow": qrow,
            "biasv": bias.reshape(1, HC),
        })
    return in_maps


def assemble(results):
    out = np.empty((B, N, HC), dtype=np.float32)
    for c in range(N_CORES):
        b, half = c // 2, c % 2
        out[b, half * ID:(half + 1) * ID, :] = results[c]["out"].astype(np.float32)
    return out


def kernel(x, adj, W, att_src, att_dst, bias):
    has_bias = bool(np.any(np.asarray(bias)))
    nc = _get_nc(1, has_bias)
    in_maps = make_in_maps(x, adj, W, att_src, att_dst, bias)
    res = run_bass_kernel_spmd(nc, in_maps, list(range(N_CORES)))
    return assemble(res.results)


# revision 35
# speedup vs baseline: 1.1804x; 1.1804x over previous
"""DenseGATConv Bass/Tile kernel for Trainium2, SPMD over 8 NeuronCores.

Problem (B=4, N=2048, F=128, H=4, C=64):
  xh = (x @ W).reshape(B,N,H,C)
  a_src[b,j,h] = xh . att_src ; a_dst[b,i,h] = xh . att_dst
  s = a_src[j] + a_dst[i];  alpha = softmax_j(mask(adj+I, leaky_relu(s, 0.2)))
  out[b,i] = concat_h(sum_j alpha * xh[b,j,h,:]) + bias

Algebra (no exp over the N*N*H grid, no softmax-normalizer subtraction):
  exp(lrelu(s)) = max(exp(s), exp(0.2 s));  dividing by exp(a_dst_i)
  (cancels in softmax) gives the grid weight
    T[j,h,i] = max(Q'_i * e2_j, e1_j),  Q' = exp(-0.8 a_dst),
    e2 = exp(0.2 a_src), e1 = exp(a_src)
  G = T * adjT (self loops host-added).  PE accumulates num|den with one
  stationary load per (tile, head):
    acc[h][c,i] += xh1[j, c|1]^T @ G[j, h, i]   (f16, f32 PSUM)
  row 64 of acc is the softmax denominator (ones column in xh1).

All grid elementwise work runs on DVE (T ops ~0.45ns/elem, mask-mults
~0.55ns/elem, both in the f16 2x mode); offloading mask-mults to the
Pool engine was measured to SLOW both engines ~2.4x (shared SBUF
ports), so POOL_HEADS defaults to 0,0.  Phase A is f16 end to end (PE 1
cycle/row instead of 4).  The per-core q slice is
made core-independent by rolling each core's own 1024 destination
columns to the front of xT (and reordering adjT rows identically - the
contraction order is internal).  The softmax scalars (e1/e2 per source
node, Q' per destination) are host-folded like the weight folds: expv
is a 64KB input and Q' arrives as an 8KB row that a PE ones
outer-product broadcasts to all partitions, so the grid stream gates
only on small leading DMAs.  The destination permutation (i' = kk*128+p
<-> i = p*4+kk within 512-blocks) is applied host-side to qrow/adjT, so
the output DMA writes 4 consecutive rows per partition.

Epilogue: evacuate PSUM (ACT/DVE alternating), PE-transpose [65,128]
blocks, reciprocal + one multiply per (k,h) on DVE, f16 output DMA
(host upcasts).  Bias ops are only emitted when bias is nonzero.
"""

import os

import numpy as np

import concourse.bacc as bacc
import concourse.bass as bass
import concourse.tile as tile
from concourse import mybir
from concourse.bass_utils import run_bass_kernel_spmd
from concourse.masks import make_identity

B, N, F = 4, 2048, 128
H, C = 4, 64
HC = H * C
TBUFS = int(os.environ.get('TBUFS', 5))
GBUFS = int(os.environ.get('GBUFS', 6))
ABUFS = int(os.environ.get('ABUFS', 3))
POOL_HEADS = os.environ.get('POOL_HEADS', '1,2')  # heads for pool on even,odd tiles
N_CORES = 8
ID = N // 2          # dest rows per core
NT = N // 128        # 16 source tiles
NKD = ID // 512      # 2 dest 512-chunks
F32 = mybir.dt.float32
F16 = mybir.dt.float16

_NC_CACHE = {}


def build_nc(reps: int = 1, has_bias: bool = False):
    nc = bacc.Bacc("TRN2", target_bir_lowering=False, debug=False, num_devices=1)

    d_xT = nc.dram_tensor("xT", [F, N], F16, kind="ExternalInput").ap()
    d_adjT = nc.dram_tensor("adjT", [NT, 128, ID], F16, kind="ExternalInput").ap()
    d_wcat = nc.dram_tensor("Wcat", [F, HC], F16, kind="ExternalInput").ap()
    d_expv = nc.dram_tensor("expvin", [128, NT, 8], F32, kind="ExternalInput").ap()
    d_qrow = nc.dram_tensor("qrow", [1, H * ID], F16, kind="ExternalInput").ap()
    d_bias = nc.dram_tensor("biasv", [1, HC], F32, kind="ExternalInput").ap()
    d_out = nc.dram_tensor("out", [ID, HC], F16, kind="ExternalOutput").ap()

    EXP = mybir.ActivationFunctionType.Exp
    CPY = mybir.ActivationFunctionType.Copy

    pool_even_n = int(POOL_HEADS.split(',')[0])
    pool_odd_n = int(POOL_HEADS.split(',')[1])

    with tile.TileContext(nc) as tc:
        with tc.tile_pool(name="const", bufs=1) as const:
            ident = const.tile([128, 128], F32)
            make_identity(nc, ident)
            ones1 = const.tile([1, 128], F32)
            nc.vector.memset(ones1, 1.0)

            # preload the activation table set while input DMAs run
            scratch1 = const.tile([1, 4], F32)
            nc.scalar.activation(scratch1, ones1[0:1, 0:4], CPY)
            ones1h = const.tile([1, 128], F16)
            nc.vector.memset(ones1h, 1.0)

            xT = const.tile([F, N], F16)
            wcat = const.tile([F, HC], F16)
            qrow_sb = const.tile([1, H * ID], F16)
            expv = const.tile([128, NT, 8], F32)
            q_bc = const.tile([128, H, ID], F16)        # Q' broadcast per head
            nc.sync.dma_start(out=qrow_sb, in_=d_qrow)
            nc.sync.dma_start(out=expv, in_=d_expv)
            # heads 0/1: DMA partition-broadcast straight from the input row
            # (lands before the PE chain); heads 2/3 via PE outer-product
            for h in range(2):
                hrow = d_qrow[0:1, h * ID:(h + 1) * ID]
                q_bcast = bass.AP(
                    tensor=hrow.tensor, offset=hrow.offset,
                    ap=[[0, 128]] + list(hrow.ap[1:]))
                nc.sync.dma_start(out=q_bc[:, h, :], in_=q_bcast)
            nc.sync.dma_start(out=wcat, in_=d_wcat)
            nc.sync.dma_start(out=xT[:, 0:1024], in_=d_xT[:, 0:1024])
            nc.sync.dma_start(out=xT[:, 1024:2048], in_=d_xT[:, 1024:2048])
            bias_sb = const.tile([1, HC], F32)
            if has_bias:
                nc.sync.dma_start(out=bias_sb, in_=d_bias)

            # persistent per-core tensors
            xh1 = const.tile([128, NT, H, 65], F16)     # [xh | 1] per (t,h)
            bias_bc = const.tile([128, HC], F32)

            # ---------------- phase A: projections ----------------
            with tc.tile_pool(name="psA", bufs=2, space="PSUM") as psA, \
                 tc.tile_pool(name="psQ", bufs=2, space="PSUM") as psQp, \
                 tc.tile_pool(name="psB", bufs=1, space="PSUM") as psBp:
                sc_a = nc.enter_named_scope("phA", False)
                # grid scalars and Q' come host-precomputed.  Q' is an 8KB
                # row DMA broadcast on-chip by a PE ones outer-product, so
                # the grid start does not wait on 1MB of broadcast DMA.
                for h in range(2, H):
                    qb_ps = psQp.tile([128, ID], F32)
                    for kb in range(NKD):
                        nc.tensor.matmul(
                            qb_ps[:, kb * 512:(kb + 1) * 512], ones1h,
                            qrow_sb[0:1, h * ID + kb * 512:h * ID + (kb + 1) * 512],
                            start=True, stop=True)
                    nc.scalar.activation(q_bc[:, h, :], qb_ps, CPY)
                nc.gpsimd.memset(xh1[:, :, :, 64:65], 1.0)
                # projection tiles; grid tile t can start once tile t is done
                for t in range(NT):
                    ps = psA.tile([128, HC], F32)
                    nc.tensor.matmul(ps, xT[:, t * 128:(t + 1) * 128], wcat,
                                     start=True, stop=True)
                    # raw xh into the 65-column head blocks
                    nc.scalar.activation(xh1[:, t, :, 0:64], ps, CPY)
                if has_bias:
                    psb2 = psBp.tile([128, HC], F32, tag="psbias", bufs=1)
                    nc.tensor.matmul(psb2, ones1, bias_sb, start=True, stop=True)
                    nc.scalar.activation(bias_bc, psb2, CPY)
                nc.leave_named_scope("phA", sc_a[0], False)

            # ---------------- phase B: grid + matmul accumulate ----------------
            with tc.tile_pool(name="ep_sb", bufs=1) as epsb:
                with tc.tile_pool(name="acc", bufs=1, space="PSUM") as accp:
                    acc = {}
                    for h in range(H):
                        acc_t = accp.tile([65, ID], F32, tag=f"acc{h}",
                                          name=f"acc{h}")
                        acc[h] = acc_t

                    sc_b = nc.enter_named_scope("phB", False)
                    with tc.tile_pool(name="adj", bufs=ABUFS) as adjp, \
                         tc.tile_pool(name="grid", bufs=4) as gridp:
                        for rep in range(reps):
                            for t in range(NT):
                                adjt = adjp.tile([128, ID], F16)
                                nc.sync.dma_start(out=adjt, in_=d_adjT[t])
                                t_all = gridp.tile([128, H, ID], F16, tag="T", bufs=TBUFS)
                                g = gridp.tile([128, H, ID], F16, tag="G", bufs=GBUFS)
                                adj_rep2 = bass.AP(
                                    tensor=adjt.tensor, offset=adjt.offset,
                                    ap=[adjt.ap[0], [0, 2]] + list(adjt.ap[1:]))
                                first = (rep == 0 and t == 0)
                                last = (rep == reps - 1 and t == NT - 1)
                                # head pairs: ts,ts,tt then the pair's matmuls,
                                # so PE starts 2 heads earlier on each tile
                                for hp2 in range(2):
                                    for h in (2 * hp2, 2 * hp2 + 1):
                                        # T = max(Q' * exp(.2 a_src), exp(a_src))
                                        nc.vector.tensor_scalar(
                                            out=t_all[:, h, :], in0=q_bc[:, h, :],
                                            scalar1=expv[:, t, h:h + 1],
                                            scalar2=expv[:, t, 4 + h:5 + h],
                                            op0=mybir.AluOpType.mult,
                                            op1=mybir.AluOpType.max)
                                    nc.vector.tensor_tensor(
                                        out=g[:, 2 * hp2:2 * hp2 + 2, :],
                                        in0=t_all[:, 2 * hp2:2 * hp2 + 2, :],
                                        in1=adj_rep2, op=mybir.AluOpType.mult)
                                    for h in (2 * hp2, 2 * hp2 + 1):
                                        for k in range(NKD):
                                            nc.tensor.matmul(
                                                acc[h][:, k * 512:(k + 1) * 512],
                                                xh1[:, t, h, :],
                                                g[:, h, k * 512:(k + 1) * 512],
                                                start=first, stop=last)

                    nc.leave_named_scope("phB", sc_b[0], False)
                    sc_c = nc.enter_named_scope("phC", False)
                    # evacuate accumulators to SBUF (ACT is close to PSUM)
                    s_tiles = {}
                    for h in range(H):
                        s = epsb.tile([65, ID], F32, tag=f"s{h}", name=f"s{h}")
                        if h % 2 == 0:
                            nc.scalar.activation(s, acc[h], CPY)
                        else:
                            nc.vector.tensor_copy(s, acc[h])
                        for k in range(NKD):
                            s_tiles[(h, k)] = s[:, k * 512:(k + 1) * 512]

                # acc PSUM released here
                # ------------- phase C: transpose + divide (+bias) + out -------------
                with tc.tile_pool(name="ep_ps", bufs=4, space="PSUM") as epps, \
                     tc.tile_pool(name="ep_sm", bufs=8) as epsm, \
                     tc.tile_pool(name="outp", bufs=2) as outp:
                    for k in range(NKD):
                        osb = outp.tile([128, 4, HC], F16, tag="osb", name="osb")
                        for h in range(H):
                            pt = epps.tile([128, 4, 65], F32)
                            for kk in range(4):
                                nc.tensor.transpose(
                                    pt[:, kk, :],
                                    s_tiles[(h, k)][:, kk * 128:(kk + 1) * 128],
                                    ident[0:65, 0:65])
                            rec = epsm.tile([128, 4, 1], F32)
                            nc.vector.reciprocal(rec, pt[:, :, 64:65])
                            rec_rep = bass.AP(
                                tensor=rec.tensor, offset=rec.offset,
                                ap=[rec.ap[0], rec.ap[1], [0, 64]])
                            nc.vector.tensor_tensor(
                                out=osb[:, :, h * 64:(h + 1) * 64],
                                in0=pt[:, :, 0:64], in1=rec_rep,
                                op=mybir.AluOpType.mult)
                            if has_bias:
                                bias_rep = bass.AP(
                                    tensor=bias_bc.tensor,
                                    offset=bias_bc.offset + h * 64,
                                    ap=[bias_bc.ap[0], [0, 4], [1, 64]])
                                nc.vector.tensor_tensor(
                                    out=osb[:, :, h * 64:(h + 1) * 64],
                                    in0=osb[:, :, h * 64:(h + 1) * 64],
                                    in1=bias_rep,
                                    op=mybir.AluOpType.add)
                        # destination rows are host-permuted so partition p
                        # holds 4 consecutive output rows (contiguous 2KB f16
                        # descriptor per partition)
                        blk = d_out[k * 512:(k + 1) * 512, :]
                        out_ap = bass.AP(
                            tensor=blk.tensor, offset=blk.offset,
                            ap=[[4 * HC, 128], [HC, 4], [1, HC]])
                        nc.sync.dma_start(out=out_ap, in_=osb)
                    nc.leave_named_scope("phC", sc_c[0], False)

    nc.compile()
    return nc


def _get_nc(reps: int = 1, has_bias: bool = False):
    key = (reps, has_bias)
    if key not in _NC_CACHE:
        _NC_CACHE[key] = build_nc(reps, has_bias)
    return _NC_CACHE[key]


def make_in_maps(x, adj, W, att_src, att_dst, bias):
    x = np.asarray(x, dtype=np.float32)
    adj = np.asarray(adj, dtype=np.float32)
    W = np.asarray(W, dtype=np.float32)
    att_src = np.asarray(att_src, dtype=np.float32)
    att_dst = np.asarray(att_dst, dtype=np.float32)
    bias = np.asarray(bias, dtype=np.float32)

    # weight prep: fold per-head attention dots into projection columns
    wa_src = np.stack([W[:, h * C:(h + 1) * C] @ att_src[h] for h in range(H)], 1)
    wa_dst = np.stack([W[:, h * C:(h + 1) * C] @ att_dst[h] for h in range(H)], 1)
    wcat = np.ascontiguousarray(W, dtype=np.float16)               # [F, 256]

    adjl = adj.copy()
    idx = np.arange(N)
    adjl[:, idx, idx] = 1.0

    # destination-row permutation: kernel position i' = kk*128 + p within each
    # 512-block maps to original row p*4 + kk, so the output DMA writes
    # contiguous per-partition chunks (the q-path moving AP applies the same
    # permutation on-device)
    perm = np.concatenate([kb * 512 + (np.arange(512) % 128) * 4 + np.arange(512) // 128
                           for kb in range(ID // 512)])

    in_maps = []
    for c in range(N_CORES):
        b, half = c // 2, c % 2
        # roll the core's own destination columns to the front so the shared
        # program's q slice (cols 0:ID) is core-independent; reorder adjT's
        # source rows identically (contraction order is internal)
        order = np.concatenate([np.arange(half * ID, (half + 1) * ID),
                                np.arange((1 - half) * ID, (2 - half) * ID)])
        xT = np.ascontiguousarray(x[b].T[:, order], dtype=np.float16)
        adjT = np.ascontiguousarray(
            adjl[b].T[order, :][:, half * ID + perm]).astype(np.float16)
        # softmax scalars, host-folded: e2|e1 per source node (rolled order),
        # Q' per destination node (permuted order)
        a_src = (x[b] @ wa_src)[order]
        expvin = np.concatenate([np.exp(0.2 * a_src), np.exp(a_src)],
                                axis=1).astype(np.float32)
        expvin = np.ascontiguousarray(
            expvin.reshape(NT, 128, 8).transpose(1, 0, 2))
        a_dst = (x[b] @ wa_dst)[half * ID + perm]
        qrow = np.ascontiguousarray(
            np.exp(-0.8 * a_dst).T.astype(np.float16)).reshape(1, H * ID)
        in_maps.append({
            "xT": xT,
            "adjT": adjT.reshape(NT, 128, ID),
            "Wcat": wcat,
            "expvin": expvin,
            "qrow": qrow,
            "biasv": bias.reshape(1, HC),
        })
    return in_maps


def assemble(results):
    out = np.empty((B, N, HC), dtype=np.float32)
    for c in range(N_CORES):
        b, half = c // 2, c % 2
        out[b, half * ID:(half + 1) * ID, :] = results[c]["out"].astype(np.float32)
    return out


def kernel(x, adj, W, att_src, att_dst, bias):
    has_bias = bool(np.any(np.asarray(bias)))
    nc = _get_nc(1, has_bias)
    in_maps = make_in_maps(x, adj, W, att_src, att_dst, bias)
    res = run_bass_kernel_spmd(nc, in_maps, list(range(N_CORES)))
    return assemble(res.results)


# revision 37
# speedup vs baseline: 1.1846x; 1.0036x over previous
"""DenseGATConv Bass/Tile kernel for Trainium2, SPMD over 8 NeuronCores.

Problem (B=4, N=2048, F=128, H=4, C=64):
  xh = (x @ W).reshape(B,N,H,C)
  a_src[b,j,h] = xh . att_src ; a_dst[b,i,h] = xh . att_dst
  s = a_src[j] + a_dst[i];  alpha = softmax_j(mask(adj+I, leaky_relu(s, 0.2)))
  out[b,i] = concat_h(sum_j alpha * xh[b,j,h,:]) + bias

Algebra (no exp over the N*N*H grid, no softmax-normalizer subtraction):
  exp(lrelu(s)) = max(exp(s), exp(0.2 s));  dividing by exp(a_dst_i)
  (cancels in softmax) gives the grid weight
    T[j,h,i] = max(Q'_i * e2_j, e1_j),  Q' = exp(-0.8 a_dst),
    e2 = exp(0.2 a_src), e1 = exp(a_src)
  G = T * adjT (self loops host-added).  PE accumulates num|den with one
  stationary load per (tile, head):
    acc[h][c,i] += xh1[j, c|1]^T @ G[j, h, i]   (f16, f32 PSUM)
  row 64 of acc is the softmax denominator (ones column in xh1).

All grid elementwise work runs on DVE (T ops ~0.45ns/elem, mask-mults
~0.55ns/elem, both in the f16 2x mode); offloading mask-mults to the
Pool engine was measured to SLOW both engines ~2.4x (shared SBUF
ports), so POOL_HEADS defaults to 0,0.  Phase A is f16 end to end (PE 1
cycle/row instead of 4).  The per-core q slice is
made core-independent by rolling each core's own 1024 destination
columns to the front of xT (and reordering adjT rows identically - the
contraction order is internal).  The softmax scalars (e1/e2 per source
node, Q' per destination) are host-folded like the weight folds: expv
is a 64KB input and Q' arrives as an 8KB row that a PE ones
outer-product broadcasts to all partitions, so the grid stream gates
only on small leading DMAs.  The destination permutation (i' = kk*128+p
<-> i = p*4+kk within 512-blocks) is applied host-side to qrow/adjT, so
the output DMA writes 4 consecutive rows per partition.

Epilogue: evacuate PSUM (ACT/DVE alternating), PE-transpose [65,128]
blocks, reciprocal + one multiply per (k,h) on DVE, f16 output DMA
(host upcasts).  Bias ops are only emitted when bias is nonzero.
"""

import os

import numpy as np

import concourse.bacc as bacc
import concourse.bass as bass
import concourse.tile as tile
from concourse import mybir
from concourse.bass_utils import run_bass_kernel_spmd
from concourse.masks import make_identity

B, N, F = 4, 2048, 128
H, C = 4, 64
HC = H * C
TBUFS = int(os.environ.get('TBUFS', 5))
GBUFS = int(os.environ.get('GBUFS', 6))
ABUFS = int(os.environ.get('ABUFS', 3))
POOL_HEADS = os.environ.get('POOL_HEADS', '1,2')  # heads for pool on even,odd tiles
N_CORES = 8
ID = N // 2          # dest rows per core
NT = N // 128        # 16 source tiles
NKD = ID // 512      # 2 dest 512-chunks
F32 = mybir.dt.float32
F16 = mybir.dt.float16

_NC_CACHE = {}


def build_nc(reps: int = 1, has_bias: bool = False):
    nc = bacc.Bacc("TRN2", target_bir_lowering=False, debug=False, num_devices=1)

    d_xT = nc.dram_tensor("xT", [F, N], F16, kind="ExternalInput").ap()
    d_adjT = nc.dram_tensor("adjT", [NT, 128, ID], F16, kind="ExternalInput").ap()
    d_wcat = nc.dram_tensor("Wcat", [F, HC], F16, kind="ExternalInput").ap()
    d_expv = nc.dram_tensor("expvin", [128, NT, 8], F32, kind="ExternalInput").ap()
    d_qrow = nc.dram_tensor("qrow", [1, H * ID], F16, kind="ExternalInput").ap()
    d_bias = nc.dram_tensor("biasv", [1, HC], F32, kind="ExternalInput").ap()
    d_out = nc.dram_tensor("out", [ID, HC], F16, kind="ExternalOutput").ap()

    EXP = mybir.ActivationFunctionType.Exp
    CPY = mybir.ActivationFunctionType.Copy

    pool_even_n = int(POOL_HEADS.split(',')[0])
    pool_odd_n = int(POOL_HEADS.split(',')[1])

    with tile.TileContext(nc) as tc:
        with tc.tile_pool(name="const", bufs=1) as const:
            ident = const.tile([128, 128], F32)
            make_identity(nc, ident)
            ones1 = const.tile([1, 128], F32)
            nc.vector.memset(ones1, 1.0)

            # preload the activation table set while input DMAs run
            scratch1 = const.tile([1, 4], F32)
            nc.scalar.activation(scratch1, ones1[0:1, 0:4], CPY)
            ones1h = const.tile([1, 128], F16)
            nc.vector.memset(ones1h, 1.0)

            xT = const.tile([F, N], F16)
            wcat = const.tile([F, HC], F16)
            qrow_sb = const.tile([1, H * ID], F16)
            expv = const.tile([128, NT, 8], F32)
            q_bc = const.tile([128, H, ID], F16)        # Q' broadcast per head
            nc.sync.dma_start(out=qrow_sb, in_=d_qrow)
            nc.sync.dma_start(out=expv, in_=d_expv)
            # heads 0/1: DMA partition-broadcast straight from the input row
            # (lands before the PE chain); heads 2/3 via PE outer-product
            for h in range(2):
                hrow = d_qrow[0:1, h * ID:(h + 1) * ID]
                q_bcast = bass.AP(
                    tensor=hrow.tensor, offset=hrow.offset,
                    ap=[[0, 128]] + list(hrow.ap[1:]))
                nc.sync.dma_start(out=q_bc[:, h, :], in_=q_bcast)
            nc.sync.dma_start(out=wcat, in_=d_wcat)
            nc.sync.dma_start(out=xT[:, 0:1024], in_=d_xT[:, 0:1024])
            nc.sync.dma_start(out=xT[:, 1024:2048], in_=d_xT[:, 1024:2048])
            bias_sb = const.tile([1, HC], F32)
            if has_bias:
                nc.sync.dma_start(out=bias_sb, in_=d_bias)

            # persistent per-core tensors
            xh1 = const.tile([128, NT, H, 65], F16)     # [xh | 1] per (t,h)
            bias_bc = const.tile([128, HC], F32)

            # ---------------- phase A: projections ----------------
            with tc.tile_pool(name="psA", bufs=2, space="PSUM") as psA, \
                 tc.tile_pool(name="psQ", bufs=2, space="PSUM") as psQp, \
                 tc.tile_pool(name="psB", bufs=1, space="PSUM") as psBp:
                sc_a = nc.enter_named_scope("phA", False)
                # grid scalars and Q' come host-precomputed.  Q' is an 8KB
                # row DMA broadcast on-chip by a PE ones outer-product, so
                # the grid start does not wait on 1MB of broadcast DMA.
                for h in range(2, H):
                    qb_ps = psQp.tile([128, ID], F32)
                    for kb in range(NKD):
                        nc.tensor.matmul(
                            qb_ps[:, kb * 512:(kb + 1) * 512], ones1h,
                            qrow_sb[0:1, h * ID + kb * 512:h * ID + (kb + 1) * 512],
                            start=True, stop=True)
                    nc.scalar.activation(q_bc[:, h, :], qb_ps, CPY)
                nc.gpsimd.memset(xh1[:, :, :, 64:65], 1.0)
                # projection tiles; grid tile t can start once tile t is done
                for t in range(NT):
                    ps = psA.tile([128, HC], F32)
                    nc.tensor.matmul(ps, xT[:, t * 128:(t + 1) * 128], wcat,
                                     start=True, stop=True)
                    # raw xh into the 65-column head blocks
                    nc.scalar.activation(xh1[:, t, :, 0:64], ps, CPY)
                if has_bias:
                    psb2 = psBp.tile([128, HC], F32, tag="psbias", bufs=1)
                    nc.tensor.matmul(psb2, ones1, bias_sb, start=True, stop=True)
                    nc.scalar.activation(bias_bc, psb2, CPY)
                nc.leave_named_scope("phA", sc_a[0], False)

            # ---------------- phase B: grid + matmul accumulate ----------------
            with tc.tile_pool(name="ep_sb", bufs=1) as epsb:
                with tc.tile_pool(name="acc", bufs=1, space="PSUM") as accp:
                    acc = {}
                    for h in range(H):
                        acc_t = accp.tile([65, ID], F32, tag=f"acc{h}",
                                          name=f"acc{h}")
                        acc[h] = acc_t

                    sc_b = nc.enter_named_scope("phB", False)
                    with tc.tile_pool(name="adj", bufs=ABUFS) as adjp, \
                         tc.tile_pool(name="grid", bufs=4) as gridp:
                        for rep in range(reps):
                            for t in range(NT):
                                adjt = adjp.tile([128, ID], F16)
                                nc.sync.dma_start(out=adjt, in_=d_adjT[t])
                                t_all = gridp.tile([128, H, ID], F16, tag="T", bufs=TBUFS)
                                g = gridp.tile([128, H, ID], F16, tag="G", bufs=GBUFS)
                                adj_rep2 = bass.AP(
                                    tensor=adjt.tensor, offset=adjt.offset,
                                    ap=[adjt.ap[0], [0, 2]] + list(adjt.ap[1:]))
                                first = (rep == 0 and t == 0)
                                last = (rep == reps - 1 and t == NT - 1)
                                # head pairs: ts,ts,tt then the pair's matmuls,
                                # so PE starts 2 heads earlier on each tile
                                for hp2 in range(2):
                                    for h in (2 * hp2, 2 * hp2 + 1):
                                        # T = max(Q' * exp(.2 a_src), exp(a_src))
                                        nc.vector.tensor_scalar(
                                            out=t_all[:, h, :], in0=q_bc[:, h, :],
                                            scalar1=expv[:, t, h:h + 1],
                                            scalar2=expv[:, t, 4 + h:5 + h],
                                            op0=mybir.AluOpType.mult,
                                            op1=mybir.AluOpType.max)
                                    nc.vector.tensor_tensor(
                                        out=g[:, 2 * hp2:2 * hp2 + 2, :],
                                        in0=t_all[:, 2 * hp2:2 * hp2 + 2, :],
                                        in1=adj_rep2, op=mybir.AluOpType.mult)
                                    for h in (2 * hp2, 2 * hp2 + 1):
                                        for k in range(NKD):
                                            nc.tensor.matmul(
                                                acc[h][:, k * 512:(k + 1) * 512],
                                                xh1[:, t, h, :],
                                                g[:, h, k * 512:(k + 1) * 512],
                                                start=first, stop=last)

                    nc.leave_named_scope("phB", sc_b[0], False)
                    sc_c = nc.enter_named_scope("phC", False)
                    # evacuate accumulators to SBUF (ACT is close to PSUM)
                    s_tiles = {}
                    for h in range(H):
                        s = epsb.tile([65, ID], F32, tag=f"s{h}", name=f"s{h}")
                        if h % 2 == 0:
                            nc.scalar.activation(s, acc[h], CPY)
                        else:
                            nc.vector.tensor_copy(s, acc[h])
                        for k in range(NKD):
                            s_tiles[(h, k)] = s[:, k * 512:(k + 1) * 512]

                # acc PSUM released here
                # ------------- phase C: transpose + divide (+bias) + out -------------
                with tc.tile_pool(name="ep_ps", bufs=4, space="PSUM") as epps, \
                     tc.tile_pool(name="ep_sm", bufs=8) as epsm, \
                     tc.tile_pool(name="outp", bufs=2) as outp:
                    for k in range(NKD):
                        osb = outp.tile([128, 4, HC], F16, tag="osb", name="osb")
                        for h in range(H):
                            pt = epps.tile([128, 4, 65], F32)
                            for kk in range(4):
                                nc.tensor.transpose(
                                    pt[:, kk, :],
                                    s_tiles[(h, k)][:, kk * 128:(kk + 1) * 128],
                                    ident[0:65, 0:65])
                            rec = epsm.tile([128, 4, 1], F32)
                            nc.vector.reciprocal(rec, pt[:, :, 64:65])
                            rec_rep = bass.AP(
                                tensor=rec.tensor, offset=rec.offset,
                                ap=[rec.ap[0], rec.ap[1], [0, 64]])
                            nc.vector.tensor_tensor(
                                out=osb[:, :, h * 64:(h + 1) * 64],
                                in0=pt[:, :, 0:64], in1=rec_rep,
                                op=mybir.AluOpType.mult)
                            if has_bias:
                                bias_rep = bass.AP(
                                    tensor=bias_bc.tensor,
                                    offset=bias_bc.offset + h * 64,
                                    ap=[bias_bc.ap[0], [0, 4], [1, 64]])
                                nc.vector.tensor_tensor(
                                    out=osb[:, :, h * 64:(h + 1) * 64],
                                    in0=osb[:, :, h * 64:(h + 1) * 64],
                                    in1=bias_rep,
                                    op=mybir.AluOpType.add)
                        # destination rows are host-permuted so partition p
                        # holds 4 consecutive output rows (contiguous 2KB f16
                        # descriptor per partition)
                        blk = d_out[k * 512:(k + 1) * 512, :]
                        out_ap = bass.AP(
                            tensor=blk.tensor, offset=blk.offset,
                            ap=[[4 * HC, 128], [HC, 4], [1, HC]])
                        nc.sync.dma_start(out=out_ap, in_=osb)
                    nc.leave_named_scope("phC", sc_c[0], False)

    nc.compile()
    return nc


def _get_nc(reps: int = 1, has_bias: bool = False):
    key = (reps, has_bias)
    if key not in _NC_CACHE:
        _NC_CACHE[key] = build_nc(reps, has_bias)
    return _NC_CACHE[key]


def make_in_maps(x, adj, W, att_src, att_dst, bias):
    x = np.asarray(x, dtype=np.float32)
    adj = np.asarray(adj, dtype=np.float32)
    W = np.asarray(W, dtype=np.float32)
    att_src = np.asarray(att_src, dtype=np.float32)
    att_dst = np.asarray(att_dst, dtype=np.float32)
    bias = np.asarray(bias, dtype=np.float32)

    # weight prep: fold per-head attention dots into projection columns
    wa_src = np.stack([W[:, h * C:(h + 1) * C] @ att_src[h] for h in range(H)], 1)
    wa_dst = np.stack([W[:, h * C:(h + 1) * C] @ att_dst[h] for h in range(H)], 1)
    wcat = np.ascontiguousarray(W, dtype=np.float16)               # [F, 256]

    adjl = adj.copy()
    idx = np.arange(N)
    adjl[:, idx, idx] = 1.0

    # destination-row permutation: kernel position i' = kk*128 + p within each
    # 512-block maps to original row p*4 + kk, so the output DMA writes
    # contiguous per-partition chunks (the q-path moving AP applies the same
    # permutation on-device)
    perm = np.concatenate([kb * 512 + (np.arange(512) % 128) * 4 + np.arange(512) // 128
                           for kb in range(ID // 512)])

    in_maps = []
    for c in range(N_CORES):
        b, half = c // 2, c % 2
        # roll the core's own destination columns to the front so the shared
        # program's q slice (cols 0:ID) is core-independent; reorder adjT's
        # source rows identically (contraction order is internal)
        order = np.concatenate([np.arange(half * ID, (half + 1) * ID),
                                np.arange((1 - half) * ID, (2 - half) * ID)])
        xT = np.ascontiguousarray(x[b].T[:, order], dtype=np.float16)
        adjT = np.ascontiguousarray(
            adjl[b].T[order, :][:, half * ID + perm]).astype(np.float16)
        # softmax scalars, host-folded: e2|e1 per source node (rolled order),
        # Q' per destination node (permuted order)
        a_src = (x[b] @ wa_src)[order]
        expvin = np.concatenate([np.exp(0.2 * a_src), np.exp(a_src)],
                                axis=1).astype(np.float32)
        expvin = np.ascontiguousarray(
            expvin.reshape(NT, 128, 8).transpose(1, 0, 2))
        a_dst = (x[b] @ wa_dst)[half * ID + perm]
        qrow = np.ascontiguousarray(
            np.exp(-0.8 * a_dst).T.astype(np.float16)).reshape(1, H * ID)
        in_maps.append({
            "xT": xT,
            "adjT": adjT.reshape(NT, 128, ID),
            "Wcat": wcat,
            "expvin": expvin,
            "qrow": qrow,
            "biasv": bias.reshape(1, HC),
        })
    return in_maps


def assemble(results):
    out = np.empty((B, N, HC), dtype=np.float32)
    for c in range(N_CORES):
        b, half = c // 2, c % 2
        out[b, half * ID:(half + 1) * ID, :] = results[c]["out"].astype(np.float32)
    return out


def kernel(x, adj, W, att_src, att_dst, bias):
    has_bias = bool(np.any(np.asarray(bias)))
    nc = _get_nc(1, has_bias)
    in_maps = make_in_maps(x, adj, W, att_src, att_dst, bias)
    res = run_bass_kernel_spmd(nc, in_maps, list(range(N_CORES)))
    return assemble(res.results)
